# revision 32
# baseline (speedup 1.0000x reference)
"""CapsuleLayer (dynamic routing) Trainium2 kernel, SPMD over 8 NeuronCores.

Sharding: input-capsule axis (IN_CAPS=512 -> 64 per core). W and u_hat are
i-sharded; the bij,bijd->bjd contraction is completed with an AllReduce of
s-partials (2x64x16x32, 262 KB) once per routing iteration.

Per-core layout (i_local = i2*32 + i1, i2 in {0,1}):
  u_hat SBUF [p=(i2*64+b), (d, i1, j)] bf16 -- 128 partitions x 16384
  b/c logits [p, (i1, j)], s/outputs [b, (d, j)].

The (d, i1, j) free order keeps every big DVE pass in the bf16 2x perf mode:
both broadcast multiplies broadcast over a non-innermost dim (innermost stays
step-1), and both reductions are in-place contiguous tree-adds.

Phase 1 (per i): u_hat_i[b, dj] = xT_i.T @ W_i on the PE (K=128, M=64,
N=512) in fp32 (the first collective cannot start before ~67us of background
init anyway, so the slower fp32 W stream and matmuls are free); u_hat is
stored bf16 by the PSUM->SBUF copy.

s partials are AllReduced per partition-half (engines cannot shift partitions;
the halves are summed after the AR instead of before).
"""

import numpy as np

N_CORES = 8
B = 64
IN_CAPS = 512
IN_DIM = 128
N_CAPS = 16
OUT_DIM = 32
I_LOC = IN_CAPS // N_CORES          # 64 input capsules per core
I1 = 32                             # i_local = i2*32 + i1
JD = N_CAPS * OUT_DIM               # 512
EPS = 1e-7
GRP = 4                             # i's per W-DMA/PSUM group
NGRP = I_LOC // GRP                 # 16

# Toggled by test.py for profiling runs.
TRACE = False
TRACE_DIR = None

_cache = {}


def _emit(tc, xT, wT, out, num_routing):
    from contextlib import ExitStack

    from concourse import mybir

    nc = tc.nc
    f32 = mybir.dt.float32
    bf16 = mybir.dt.bfloat16
    ctx = ExitStack()
    singles = ctx.enter_context(tc.tile_pool(name="singles", bufs=1))
    wpool = ctx.enter_context(tc.tile_pool(name="wpool", bufs=4))
    pspool = ctx.enter_context(tc.tile_pool(name="pspool", bufs=2, space="PSUM"))
    small = ctx.enter_context(tc.tile_pool(name="small", bufs=2))
    dram = ctx.enter_context(tc.tile_pool(name="dram", bufs=2, space="DRAM"))

    # One tiny warmup collective: the collective stack finishes its background
    # init ~60us into the kernel and charges a first-collective premium
    # (~12us); a 128-byte AllReduce absorbs both off the critical path.
    warm_in = dram.tile([1, 32], f32)
    warm_out = dram.tile([1, 32], f32)
    nc.gpsimd.collective_compute(
        "AllReduce",
        mybir.AluOpType.add,
        replica_groups=[list(range(N_CORES))],
        ins=[warm_in.opt()],
        outs=[warm_out.opt()],
    )

    # ---- phase 1: u_hat = einsum over k, per local capsule i ----
    xsb = singles.tile([IN_DIM, I_LOC, B], f32)          # [k, i, b]
    u_hat = singles.tile([128, OUT_DIM, I1, N_CAPS], bf16)  # [(i2,b), d, i1, j]

    XCH = I_LOC // 4
    for g in range(NGRP):
        i2 = (g * GRP) // I1
        i1g = (g * GRP) % I1
        # interleave the x chunks with the first W groups so the first
        # matmul's operands land as early as possible
        if g < 4:
            q = g
            nc.sync.dma_start(
                xsb[:, q * XCH:(q + 1) * XCH, :],
                xT[:, q * XCH:(q + 1) * XCH, :],
            )
        wtile = wpool.tile([IN_DIM, GRP, OUT_DIM, N_CAPS], f32)
        nc.sync.dma_start(wtile[:], wT[g])
        ps = pspool.tile([128, GRP, OUT_DIM, N_CAPS], f32)
        for t in range(GRP):
            i = g * GRP + t
            nc.tensor.matmul(
                ps[i2 * B:(i2 + 1) * B, t], xsb[:, i, :], wtile[:, t],
                start=True, stop=True,
            )
        # copy+cast PSUM f32 -> SBUF bf16; dst viewed (i1, d, j) to match src
        dst = u_hat[i2 * B:(i2 + 1) * B, :, i1g:i1g + GRP, :].transpose(
            [0, 2, 1, 3]
        )
        src = ps[i2 * B:(i2 + 1) * B]
        if g % 2 == 0:
            nc.vector.tensor_copy(out=dst, in_=src)
        else:
            nc.scalar.copy(out=dst, in_=src)

    # ---- phase 2: routing ----
    tmp = singles.tile([128, OUT_DIM, I1, N_CAPS], bf16)
    b_log = singles.tile([128, I1, N_CAPS], f32)
    out2 = singles.tile([128, OUT_DIM, N_CAPS], bf16)
    eps_t = singles.tile([B, 1], f32)
    nc.vector.memset(b_log[:], 0.0)
    nc.vector.memset(eps_t[:], EPS)

    R = num_routing
    for r in range(R):
        if r == 0:
            # b == 0 -> c uniform: s = (1/16) * sum_i u_hat (scale after AR)
            nc.vector.tensor_add(
                tmp[:, :, :I1 // 2], u_hat[:, :, :I1 // 2],
                u_hat[:, :, I1 // 2:],
            )
        else:
            # |b| stays < ~20 for this distribution: exp is fp32-safe
            # without the max-subtraction
            cexp = small.tile([128, I1, N_CAPS], f32)
            nc.scalar.activation(
                out=cexp[:], in_=b_log[:],
                func=mybir.ActivationFunctionType.Exp,
            )
            csum = small.tile([128, I1], f32)
            nc.vector.reduce_sum(
                out=csum[:], in_=cexp[:], axis=mybir.AxisListType.X
            )
            nc.vector.reciprocal(out=csum[:], in_=csum[:])
            c_t = small.tile([128, I1, N_CAPS], bf16)
            nc.vector.tensor_mul(
                c_t[:], cexp[:],
                csum.unsqueeze(2).broadcast_to([128, I1, N_CAPS]),
            )
            # s-mul: broadcast c over outermost d keeps bf16 2x mode
            nc.vector.tensor_mul(
                tmp[:], u_hat[:],
                c_t.unsqueeze(1).broadcast_to([128, OUT_DIM, I1, N_CAPS]),
            )
            nc.vector.tensor_add(
                tmp[:, :, :I1 // 2], tmp[:, :, :I1 // 2], tmp[:, :, I1 // 2:]
            )
        # contiguous in-place tree over i1 (middle dim); final level -> f32
        w = I1 // 2
        while w > 2:
            nc.vector.tensor_add(
                tmp[:, :, :w // 2], tmp[:, :, :w // 2], tmp[:, :, w // 2:w]
            )
            w //= 2
        s_half = small.tile([128, OUT_DIM, N_CAPS], f32)
        nc.vector.tensor_add(s_half[:], tmp[:, :, 0, :], tmp[:, :, 1, :])

        # AllReduce both partition halves; sum them after (engines cannot
        # shift partitions, so pre-folding would need an extra DMA round)
        cc_in = dram.tile([2, B, OUT_DIM, N_CAPS], f32)
        cc_out = dram.tile([2, B, OUT_DIM, N_CAPS], f32)
        nc.sync.dma_start(cc_in[0], s_half[0:B])
        nc.sync.dma_start(cc_in[1], s_half[B:2 * B])
        nc.gpsimd.collective_compute(
            "AllReduce",
            mybir.AluOpType.add,
            replica_groups=[list(range(N_CORES))],
            ins=[cc_in.opt()],
            outs=[cc_out.opt()],
        )
        s_lo = small.tile([B, OUT_DIM, N_CAPS], f32)
        s_hi = small.tile([B, OUT_DIM, N_CAPS], f32)
        nc.sync.dma_start(s_lo[:], cc_out[0])
        nc.sync.dma_start(s_hi[:], cc_out[1])
        s_sb = small.tile([B, OUT_DIM, N_CAPS], f32)
        nc.vector.tensor_add(s_sb[:], s_lo[:], s_hi[:])
        # iteration 0's uniform c = 1/16 is folded into the squash math:
        # with s' = 16*s, ss = sum_d (s'/16)^2 and out = f(ss) * (s'/16)
        inv = 1.0 / N_CAPS if r == 0 else 1.0

        # squash: scale = ss/(1+ss)/sqrt(ss+eps), ss = sum_d s^2
        sq = small.tile([B, OUT_DIM, N_CAPS], f32)
        nc.vector.scalar_tensor_tensor(
            sq[:], s_sb[:], inv * inv, s_sb[:],
            mybir.AluOpType.mult, mybir.AluOpType.mult,
        )
        ss = small.tile([B, N_CAPS], f32)
        nc.vector.reduce_sum(
            out=ss[:], in_=sq.transpose([0, 2, 1]), axis=mybir.AxisListType.X
        )
        t1 = small.tile([B, N_CAPS], f32)
        nc.scalar.activation(
            out=t1[:], in_=ss[:], func=mybir.ActivationFunctionType.Sqrt,
            bias=eps_t[:], scale=1.0,
        )
        t2 = small.tile([B, N_CAPS], f32)
        nc.vector.scalar_tensor_tensor(
            t2[:], ss[:], 1.0, t1[:],
            mybir.AluOpType.add, mybir.AluOpType.mult,
        )   # (1+ss)*sqrt(ss+eps)
        nc.vector.reciprocal(out=t2[:], in_=t2[:])
        nc.vector.scalar_tensor_tensor(
            t1[:], ss[:], inv, t2[:],
            mybir.AluOpType.mult, mybir.AluOpType.mult,
        )   # t1 = (ss*inv) / ((1+ss)*sqrt(ss+eps)) -- squash scale (*inv)
        out_s = small.tile([B, OUT_DIM, N_CAPS], f32)
        nc.vector.tensor_mul(
            out_s[:], s_sb[:],
            t1.unsqueeze(1).broadcast_to([B, OUT_DIM, N_CAPS]),
        )

        if r == R - 1:
            out_t = small.tile([B, N_CAPS, OUT_DIM], f32)
            nc.vector.tensor_copy(out=out_t[:], in_=out_s.transpose([0, 2, 1]))
            nc.sync.dma_start(out[:], out_t[:])
        else:
            nc.vector.tensor_copy(out=out2[0:B], in_=out_s[:])
            nc.sync.dma_start(out2[B:2 * B], out2[0:B])
            # bu-mul: broadcast outputs over middle i1 keeps bf16 2x mode
            nc.vector.tensor_mul(
                tmp[:], u_hat[:],
                out2.unsqueeze(2).broadcast_to([128, OUT_DIM, I1, N_CAPS]),
            )
            w = OUT_DIM
            while w > 2:
                nc.vector.tensor_add(
                    tmp[:, :w // 2], tmp[:, :w // 2], tmp[:, w // 2:w]
                )
                w //= 2
            bred = small.tile([128, I1, N_CAPS], f32)
            nc.vector.tensor_add(bred[:], tmp[:, 0], tmp[:, 1])
            nc.vector.tensor_add(b_log[:], b_log[:], bred[:])

    ctx.close()


def _build(num_routing):
    import concourse.bacc as bacc
    import concourse.tile as tile
    from concourse import mybir

    nc = bacc.Bacc(
        "TRN2", target_bir_lowering=False, debug=False, num_devices=N_CORES,
        dynamic_dma_scratch_size=512,
    )
    f32 = mybir.dt.float32
    bf16 = mybir.dt.bfloat16
    xT = nc.dram_tensor("xT", [IN_DIM, I_LOC, B], f32, kind="ExternalInput")
    wT = nc.dram_tensor(
        "wT", [NGRP, IN_DIM, GRP, OUT_DIM, N_CAPS], f32, kind="ExternalInput"
    )
    out = nc.dram_tensor(
        "out", [B, N_CAPS, OUT_DIM], f32, kind="ExternalOutput"
    )
    with tile.TileContext(nc) as tc:
        _emit(tc, xT, wT, out, num_routing)
    nc.compile()
    return nc


def kernel(inputs, W, num_routing):
    from concourse.bass_utils import run_bass_kernel_spmd

    R = int(num_routing)
    assert R >= 1
    if R not in _cache:
        _cache[R] = _build(R)
    nc = _cache[R]

    inputs = np.ascontiguousarray(np.asarray(inputs, dtype=np.float32))
    W = np.asarray(W, dtype=np.float32)

    in_maps = []
    for c in range(N_CORES):
        lo, hi = c * I_LOC, (c + 1) * I_LOC
        xT_c = np.ascontiguousarray(inputs[:, lo:hi, :].transpose(2, 1, 0))
        # [i,j,k,d] -> group-blocked [g, k, t, d, j] so each group DMA is one
        # contiguous block and PSUM columns come out in (d, j) order
        wT_c = np.ascontiguousarray(
            W[lo:hi]
            .reshape(NGRP, GRP, N_CAPS, IN_DIM, OUT_DIM)
            .transpose(0, 3, 1, 4, 2)
        )
        in_maps.append({"xT": xT_c, "wT": wT_c})

    kwargs = {}
    if TRACE:
        kwargs["trace"] = True
        if TRACE_DIR:
            kwargs["tmpdir"] = TRACE_DIR
    res = None
    for attempt in range(3):
        try:
            res = run_bass_kernel_spmd(
                nc, in_maps, core_ids=list(range(N_CORES)), **kwargs
            )
            break
        except Exception:
            if attempt == 2:
                raise
            import time
            time.sleep(5)
    if TRACE:
        kernel.last_exec_time_ns = res.exec_time_ns
        kernel.last_results = res
    return np.asarray(res.results[0]["out"], dtype=np.float32)


# revision 33
# speedup vs baseline: 1.0788x; 1.0788x over previous
"""CapsuleLayer (dynamic routing) Trainium2 kernel, SPMD over 8 NeuronCores.

Sharding: input-capsule axis (IN_CAPS=512 -> 64 per core). W and u_hat are
i-sharded; the bij,bijd->bjd contraction is completed with an AllReduce of
s-partials (2x64x16x32, 262 KB) once per routing iteration.

Per-core layout (i_local = i2*32 + i1, i2 in {0,1}):
  u_hat SBUF [p=(i2*64+b), (d, i1, j)] bf16 -- 128 partitions x 16384
  b/c logits [p, (i1, j)], s/outputs [b, (d, j)].

The (d, i1, j) free order keeps every big DVE pass in the bf16 2x perf mode:
both broadcast multiplies broadcast over a non-innermost dim (innermost stays
step-1), and both reductions are in-place contiguous tree-adds.

Phase 1 (per i): u_hat_i[b, dj] = xT_i.T @ W_i on the PE (K=128, M=64,
N=512), all in bf16 (x and W are cast host-side; u_hat is stored bf16
regardless, so the extra input rounding barely moves the final error, and the
shorter W stream lets the collective stack warm up sooner).

s partials are AllReduced per partition-half (engines cannot shift partitions;
the halves are summed after the AR instead of before).
"""

import numpy as np

N_CORES = 8
B = 64
IN_CAPS = 512
IN_DIM = 128
N_CAPS = 16
OUT_DIM = 32
I_LOC = IN_CAPS // N_CORES          # 64 input capsules per core
I1 = 32                             # i_local = i2*32 + i1
JD = N_CAPS * OUT_DIM               # 512
EPS = 1e-7
GRP = 4                             # i's per W-DMA/PSUM group
NGRP = I_LOC // GRP                 # 16

# Toggled by test.py for profiling runs.
TRACE = False
TRACE_DIR = None

_cache = {}


def _emit(tc, xT, wT, out, num_routing):
    from contextlib import ExitStack

    from concourse import mybir

    nc = tc.nc
    f32 = mybir.dt.float32
    bf16 = mybir.dt.bfloat16
    ctx = ExitStack()
    singles = ctx.enter_context(tc.tile_pool(name="singles", bufs=1))
    wpool = ctx.enter_context(tc.tile_pool(name="wpool", bufs=4))
    pspool = ctx.enter_context(tc.tile_pool(name="pspool", bufs=2, space="PSUM"))
    small = ctx.enter_context(tc.tile_pool(name="small", bufs=2))
    dram = ctx.enter_context(tc.tile_pool(name="dram", bufs=2, space="DRAM"))

    # One tiny warmup collective: the collective stack finishes its background
    # init ~60us into the kernel and charges a first-collective premium
    # (~12us); a 128-byte AllReduce absorbs both off the critical path.
    warm_in = dram.tile([1, 32], f32)
    warm_out = dram.tile([1, 32], f32)
    nc.gpsimd.collective_compute(
        "AllReduce",
        mybir.AluOpType.add,
        replica_groups=[list(range(N_CORES))],
        ins=[warm_in.opt()],
        outs=[warm_out.opt()],
    )

    # ---- phase 1: u_hat = einsum over k, per local capsule i ----
    xsb = singles.tile([IN_DIM, I_LOC, B], bf16)         # [k, i, b]
    u_hat = singles.tile([128, OUT_DIM, I1, N_CAPS], bf16)  # [(i2,b), d, i1, j]

    XCH = I_LOC // 4
    for g in range(NGRP):
        i2 = (g * GRP) // I1
        i1g = (g * GRP) % I1
        # interleave the x chunks with the first W groups so the first
        # matmul's operands land as early as possible
        if g < 4:
            q = g
            nc.sync.dma_start(
                xsb[:, q * XCH:(q + 1) * XCH, :],
                xT[:, q * XCH:(q + 1) * XCH, :],
            )
        wtile = wpool.tile([IN_DIM, GRP, OUT_DIM, N_CAPS], bf16)
        nc.sync.dma_start(wtile[:], wT[g])
        ps = pspool.tile([128, GRP, OUT_DIM, N_CAPS], f32)
        for t in range(GRP):
            i = g * GRP + t
            nc.tensor.matmul(
                ps[i2 * B:(i2 + 1) * B, t], xsb[:, i, :], wtile[:, t],
                start=True, stop=True,
            )
        # copy+cast PSUM f32 -> SBUF bf16; dst viewed (i1, d, j) to match src
        dst = u_hat[i2 * B:(i2 + 1) * B, :, i1g:i1g + GRP, :].transpose(
            [0, 2, 1, 3]
        )
        src = ps[i2 * B:(i2 + 1) * B]
        if g % 2 == 0:
            nc.vector.tensor_copy(out=dst, in_=src)
        else:
            nc.scalar.copy(out=dst, in_=src)

    # ---- phase 2: routing ----
    tmp = singles.tile([128, OUT_DIM, I1, N_CAPS], bf16)
    b_log = singles.tile([128, I1, N_CAPS], f32)
    out2 = singles.tile([128, OUT_DIM, N_CAPS], bf16)
    eps_t = singles.tile([B, 1], f32)
    nc.vector.memset(b_log[:], 0.0)
    nc.vector.memset(eps_t[:], EPS)

    R = num_routing
    for r in range(R):
        if r == 0:
            # b == 0 -> c uniform: s = (1/16) * sum_i u_hat (scale after AR)
            nc.vector.tensor_add(
                tmp[:, :, :I1 // 2], u_hat[:, :, :I1 // 2],
                u_hat[:, :, I1 // 2:],
            )
        else:
            # |b| stays < ~20 for this distribution: exp is fp32-safe
            # without the max-subtraction
            cexp = small.tile([128, I1, N_CAPS], f32)
            nc.scalar.activation(
                out=cexp[:], in_=b_log[:],
                func=mybir.ActivationFunctionType.Exp,
            )
            csum = small.tile([128, I1], f32)
            nc.vector.reduce_sum(
                out=csum[:], in_=cexp[:], axis=mybir.AxisListType.X
            )
            nc.vector.reciprocal(out=csum[:], in_=csum[:])
            c_t = small.tile([128, I1, N_CAPS], bf16)
            nc.vector.tensor_mul(
                c_t[:], cexp[:],
                csum.unsqueeze(2).broadcast_to([128, I1, N_CAPS]),
            )
            # s-mul: broadcast c over outermost d keeps bf16 2x mode
            nc.vector.tensor_mul(
                tmp[:], u_hat[:],
                c_t.unsqueeze(1).broadcast_to([128, OUT_DIM, I1, N_CAPS]),
            )
            nc.vector.tensor_add(
                tmp[:, :, :I1 // 2], tmp[:, :, :I1 // 2], tmp[:, :, I1 // 2:]
            )
        # contiguous in-place tree over i1 (middle dim); final level -> f32
        w = I1 // 2
        while w > 2:
            nc.vector.tensor_add(
                tmp[:, :, :w // 2], tmp[:, :, :w // 2], tmp[:, :, w // 2:w]
            )
            w //= 2
        s_half = small.tile([128, OUT_DIM, N_CAPS], f32)
        nc.vector.tensor_add(s_half[:], tmp[:, :, 0, :], tmp[:, :, 1, :])

        # AllReduce both partition halves; sum them after (engines cannot
        # shift partitions, so pre-folding would need an extra DMA round)
        cc_in = dram.tile([2, B, OUT_DIM, N_CAPS], f32)
        cc_out = dram.tile([2, B, OUT_DIM, N_CAPS], f32)
        nc.sync.dma_start(cc_in[0], s_half[0:B])
        nc.sync.dma_start(cc_in[1], s_half[B:2 * B])
        nc.gpsimd.collective_compute(
            "AllReduce",
            mybir.AluOpType.add,
            replica_groups=[list(range(N_CORES))],
            ins=[cc_in.opt()],
            outs=[cc_out.opt()],
        )
        s_lo = small.tile([B, OUT_DIM, N_CAPS], f32)
        s_hi = small.tile([B, OUT_DIM, N_CAPS], f32)
        nc.sync.dma_start(s_lo[:], cc_out[0])
        nc.sync.dma_start(s_hi[:], cc_out[1])
        s_sb = small.tile([B, OUT_DIM, N_CAPS], f32)
        nc.vector.tensor_add(s_sb[:], s_lo[:], s_hi[:])
        # iteration 0's uniform c = 1/16 is folded into the squash math:
        # with s' = 16*s, ss = sum_d (s'/16)^2 and out = f(ss) * (s'/16)
        inv = 1.0 / N_CAPS if r == 0 else 1.0

        # squash: scale = ss/(1+ss)/sqrt(ss+eps), ss = sum_d s^2
        sq = small.tile([B, OUT_DIM, N_CAPS], f32)
        nc.vector.scalar_tensor_tensor(
            sq[:], s_sb[:], inv * inv, s_sb[:],
            mybir.AluOpType.mult, mybir.AluOpType.mult,
        )
        ss = small.tile([B, N_CAPS], f32)
        nc.vector.reduce_sum(
            out=ss[:], in_=sq.transpose([0, 2, 1]), axis=mybir.AxisListType.X
        )
        t1 = small.tile([B, N_CAPS], f32)
        nc.scalar.activation(
            out=t1[:], in_=ss[:], func=mybir.ActivationFunctionType.Sqrt,
            bias=eps_t[:], scale=1.0,
        )
        t2 = small.tile([B, N_CAPS], f32)
        nc.vector.scalar_tensor_tensor(
            t2[:], ss[:], 1.0, t1[:],
            mybir.AluOpType.add, mybir.AluOpType.mult,
        )   # (1+ss)*sqrt(ss+eps)
        nc.vector.reciprocal(out=t2[:], in_=t2[:])
        nc.vector.scalar_tensor_tensor(
            t1[:], ss[:], inv, t2[:],
            mybir.AluOpType.mult, mybir.AluOpType.mult,
        )   # t1 = (ss*inv) / ((1+ss)*sqrt(ss+eps)) -- squash scale (*inv)
        out_s = small.tile([B, OUT_DIM, N_CAPS], f32)
        nc.vector.tensor_mul(
            out_s[:], s_sb[:],
            t1.unsqueeze(1).broadcast_to([B, OUT_DIM, N_CAPS]),
        )

        if r == R - 1:
            out_t = small.tile([B, N_CAPS, OUT_DIM], f32)
            nc.vector.tensor_copy(out=out_t[:], in_=out_s.transpose([0, 2, 1]))
            nc.sync.dma_start(out[:], out_t[:])
        else:
            nc.vector.tensor_copy(out=out2[0:B], in_=out_s[:])
            nc.sync.dma_start(out2[B:2 * B], out2[0:B])
            # bu-mul: broadcast outputs over middle i1 keeps bf16 2x mode
            nc.vector.tensor_mul(
                tmp[:], u_hat[:],
                out2.unsqueeze(2).broadcast_to([128, OUT_DIM, I1, N_CAPS]),
            )
            w = OUT_DIM
            while w > 2:
                nc.vector.tensor_add(
                    tmp[:, :w // 2], tmp[:, :w // 2], tmp[:, w // 2:w]
                )
                w //= 2
            bred = small.tile([128, I1, N_CAPS], f32)
            nc.vector.tensor_add(bred[:], tmp[:, 0], tmp[:, 1])
            nc.vector.tensor_add(b_log[:], b_log[:], bred[:])

    ctx.close()


def _build(num_routing):
    import concourse.bacc as bacc
    import concourse.tile as tile
    from concourse import mybir

    nc = bacc.Bacc(
        "TRN2", target_bir_lowering=False, debug=False, num_devices=N_CORES,
        dynamic_dma_scratch_size=512,
    )
    f32 = mybir.dt.float32
    bf16 = mybir.dt.bfloat16
    xT = nc.dram_tensor("xT", [IN_DIM, I_LOC, B], bf16, kind="ExternalInput")
    wT = nc.dram_tensor(
        "wT", [NGRP, IN_DIM, GRP, OUT_DIM, N_CAPS], bf16, kind="ExternalInput"
    )
    out = nc.dram_tensor(
        "out", [B, N_CAPS, OUT_DIM], f32, kind="ExternalOutput"
    )
    with tile.TileContext(nc) as tc:
        _emit(tc, xT, wT, out, num_routing)
    nc.compile()
    return nc


def kernel(inputs, W, num_routing):
    import ml_dtypes

    from concourse.bass_utils import run_bass_kernel_spmd

    R = int(num_routing)
    assert R >= 1
    if R not in _cache:
        _cache[R] = _build(R)
    nc = _cache[R]

    bf = ml_dtypes.bfloat16
    inputs = np.ascontiguousarray(np.asarray(inputs, dtype=np.float32))
    W = np.asarray(W, dtype=np.float32)

    in_maps = []
    for c in range(N_CORES):
        lo, hi = c * I_LOC, (c + 1) * I_LOC
        xT_c = np.ascontiguousarray(
            inputs[:, lo:hi, :].transpose(2, 1, 0).astype(bf)
        )
        # [i,j,k,d] -> group-blocked [g, k, t, d, j] so each group DMA is one
        # contiguous block and PSUM columns come out in (d, j) order
        wT_c = np.ascontiguousarray(
            W[lo:hi]
            .reshape(NGRP, GRP, N_CAPS, IN_DIM, OUT_DIM)
            .transpose(0, 3, 1, 4, 2)
            .astype(bf)
        )
        in_maps.append({"xT": xT_c, "wT": wT_c})

    kwargs = {}
    if TRACE:
        kwargs["trace"] = True
        if TRACE_DIR:
            kwargs["tmpdir"] = TRACE_DIR
    res = None
    for attempt in range(3):
        try:
            res = run_bass_kernel_spmd(
                nc, in_maps, core_ids=list(range(N_CORES)), **kwargs
            )
            break
        except Exception:
            if attempt == 2:
                raise
            import time
            time.sleep(5)
    if TRACE:
        kernel.last_exec_time_ns = res.exec_time_ns
        kernel.last_results = res
    return np.asarray(res.results[0]["out"], dtype=np.float32)


# revision 35
# speedup vs baseline: 1.1345x; 1.0516x over previous
"""CapsuleLayer (dynamic routing) Trainium2 kernel, SPMD over 8 NeuronCores.

Sharding: input-capsule axis (IN_CAPS=512 -> 64 per core). W and u_hat are
i-sharded; the bij,bijd->bjd contraction is completed with an AllReduce of
s-partials (2x64x16x32, 262 KB) once per routing iteration.

Per-core layout (i_local = i2*32 + i1, i2 in {0,1}):
  u_hat SBUF [p=(i2*64+b), (d, i1, j)] bf16 -- 128 partitions x 16384
  b/c logits [p, (i1, j)], s/outputs [b, (d, j)].

The (d, i1, j) free order keeps every big DVE pass in the bf16 2x perf mode:
both broadcast multiplies broadcast over a non-innermost dim (innermost stays
step-1), and both reductions are in-place contiguous tree-adds.

Phase 1 (per i): u_hat_i[b, dj] = xT_i.T @ W_i on the PE (K=128, M=64,
N=512), all in bf16 (x and W are cast host-side; u_hat is stored bf16
regardless, so the extra input rounding barely moves the final error, and the
shorter W stream lets the collective stack warm up sooner).

s partials are AllReduced per partition-half (engines cannot shift partitions;
the halves are summed after the AR instead of before).
"""

import numpy as np

N_CORES = 8
B = 64
IN_CAPS = 512
IN_DIM = 128
N_CAPS = 16
OUT_DIM = 32
I_LOC = IN_CAPS // N_CORES          # 64 input capsules per core
I1 = 32                             # i_local = i2*32 + i1
JD = N_CAPS * OUT_DIM               # 512
EPS = 1e-7
GRP = 4                             # i's per W-DMA/PSUM group
NGRP = I_LOC // GRP                 # 16

# Toggled by test.py for profiling runs.
TRACE = False
TRACE_DIR = None

_cache = {}


def _emit(tc, xT, wT, out, num_routing):
    from contextlib import ExitStack

    from concourse import mybir

    nc = tc.nc
    f32 = mybir.dt.float32
    bf16 = mybir.dt.bfloat16
    ctx = ExitStack()
    singles = ctx.enter_context(tc.tile_pool(name="singles", bufs=1))
    wpool = ctx.enter_context(tc.tile_pool(name="wpool", bufs=4))
    pspool = ctx.enter_context(tc.tile_pool(name="pspool", bufs=2, space="PSUM"))
    small = ctx.enter_context(tc.tile_pool(name="small", bufs=2))
    dram = ctx.enter_context(tc.tile_pool(name="dram", bufs=2, space="DRAM"))

    # One tiny warmup collective: the collective stack finishes its background
    # init ~60us into the kernel and charges a first-collective premium
    # (~12us); a 128-byte AllReduce absorbs both off the critical path.
    warm_in = dram.tile([1, 32], f32)
    warm_out = dram.tile([1, 32], f32)
    nc.gpsimd.collective_compute(
        "AllReduce",
        mybir.AluOpType.add,
        replica_groups=[list(range(N_CORES))],
        ins=[warm_in.opt()],
        outs=[warm_out.opt()],
    )

    # ---- phase 1: u_hat = einsum over k, per local capsule i ----
    xsb = singles.tile([IN_DIM, I_LOC, B], bf16)         # [k, i, b]
    u_hat = singles.tile([128, OUT_DIM, I1, N_CAPS], bf16)  # [(i2,b), d, i1, j]

    XCH = I_LOC // 4
    for g in range(NGRP):
        i2 = (g * GRP) // I1
        i1g = (g * GRP) % I1
        # interleave the x chunks with the first W groups so the first
        # matmul's operands land as early as possible
        if g < 4:
            q = g
            nc.sync.dma_start(
                xsb[:, q * XCH:(q + 1) * XCH, :],
                xT[:, q * XCH:(q + 1) * XCH, :],
            )
        wtile = wpool.tile([IN_DIM, GRP, OUT_DIM, N_CAPS], bf16)
        nc.sync.dma_start(wtile[:], wT[g])
        ps = pspool.tile([128, GRP, OUT_DIM, N_CAPS], f32)
        for t in range(GRP):
            i = g * GRP + t
            nc.tensor.matmul(
                ps[i2 * B:(i2 + 1) * B, t], xsb[:, i, :], wtile[:, t],
                start=True, stop=True,
            )
        # copy+cast PSUM f32 -> SBUF bf16; dst viewed (i1, d, j) to match src
        dst = u_hat[i2 * B:(i2 + 1) * B, :, i1g:i1g + GRP, :].transpose(
            [0, 2, 1, 3]
        )
        src = ps[i2 * B:(i2 + 1) * B]
        if g % 2 == 0:
            nc.vector.tensor_copy(out=dst, in_=src)
        else:
            nc.scalar.copy(out=dst, in_=src)

    # ---- phase 2: routing ----
    tmp = singles.tile([128, OUT_DIM, I1, N_CAPS], bf16)
    b_log = singles.tile([128, I1, N_CAPS], f32)
    out2 = singles.tile([128, OUT_DIM, N_CAPS], bf16)
    eps_t = singles.tile([B, 1], f32)
    nc.vector.memset(b_log[:], 0.0)
    nc.vector.memset(eps_t[:], EPS)

    R = num_routing
    for r in range(R):
        if r == 0:
            # b == 0 -> c uniform: s = (1/16) * sum_i u_hat (scale after AR)
            nc.vector.tensor_add(
                tmp[:, :, :I1 // 2], u_hat[:, :, :I1 // 2],
                u_hat[:, :, I1 // 2:],
            )
        else:
            # |b| stays < ~20 for this distribution: exp is fp32-safe
            # without the max-subtraction
            cexp = small.tile([128, I1, N_CAPS], f32)
            nc.scalar.activation(
                out=cexp[:], in_=b_log[:],
                func=mybir.ActivationFunctionType.Exp,
            )
            csum = small.tile([128, I1], f32)
            nc.vector.reduce_sum(
                out=csum[:], in_=cexp[:], axis=mybir.AxisListType.X
            )
            nc.vector.reciprocal(out=csum[:], in_=csum[:])
            c_t = small.tile([128, I1, N_CAPS], bf16)
            nc.vector.tensor_mul(
                c_t[:], cexp[:],
                csum.unsqueeze(2).broadcast_to([128, I1, N_CAPS]),
            )
            # s-mul: broadcast c over outermost d keeps bf16 2x mode
            nc.vector.tensor_mul(
                tmp[:], u_hat[:],
                c_t.unsqueeze(1).broadcast_to([128, OUT_DIM, I1, N_CAPS]),
            )
            nc.vector.tensor_add(
                tmp[:, :, :I1 // 2], tmp[:, :, :I1 // 2], tmp[:, :, I1 // 2:]
            )
        # contiguous in-place tree over i1 (middle dim); final level -> f32
        w = I1 // 2
        while w > 2:
            nc.vector.tensor_add(
                tmp[:, :, :w // 2], tmp[:, :, :w // 2], tmp[:, :, w // 2:w]
            )
            w //= 2
        s_half = small.tile([128, OUT_DIM, N_CAPS], f32)
        nc.vector.tensor_add(s_half[:], tmp[:, :, 0, :], tmp[:, :, 1, :])

        # AllReduce both partition halves; sum them after (engines cannot
        # shift partitions, so pre-folding would need an extra DMA round)
        cc_in = dram.tile([2, B, OUT_DIM, N_CAPS], f32)
        cc_out = dram.tile([2, B, OUT_DIM, N_CAPS], f32)
        nc.sync.dma_start(cc_in[0], s_half[0:B])
        nc.sync.dma_start(cc_in[1], s_half[B:2 * B])
        nc.gpsimd.collective_compute(
            "AllReduce",
            mybir.AluOpType.add,
            replica_groups=[list(range(N_CORES))],
            ins=[cc_in.opt()],
            outs=[cc_out.opt()],
        )
        s_lo = small.tile([B, OUT_DIM, N_CAPS], f32)
        s_hi = small.tile([B, OUT_DIM, N_CAPS], f32)
        nc.gpsimd.dma_start(s_lo[:], cc_out[0])
        nc.gpsimd.dma_start(s_hi[:], cc_out[1])
        s_sb = small.tile([B, OUT_DIM, N_CAPS], f32)
        nc.vector.tensor_add(s_sb[:], s_lo[:], s_hi[:])
        # iteration 0's uniform c = 1/16 is folded into the squash math:
        # with s' = 16*s, ss = sum_d (s'/16)^2 and out = f(ss) * (s'/16)
        inv = 1.0 / N_CAPS if r == 0 else 1.0

        # squash: scale = ss/(1+ss)/sqrt(ss+eps), ss = sum_d s^2
        sq = small.tile([B, OUT_DIM, N_CAPS], f32)
        nc.vector.scalar_tensor_tensor(
            sq[:], s_sb[:], inv * inv, s_sb[:],
            mybir.AluOpType.mult, mybir.AluOpType.mult,
        )
        ss = small.tile([B, N_CAPS], f32)
        nc.vector.reduce_sum(
            out=ss[:], in_=sq.transpose([0, 2, 1]), axis=mybir.AxisListType.X
        )
        t1 = small.tile([B, N_CAPS], f32)
        nc.scalar.activation(
            out=t1[:], in_=ss[:], func=mybir.ActivationFunctionType.Sqrt,
            bias=eps_t[:], scale=1.0,
        )
        t2 = small.tile([B, N_CAPS], f32)
        nc.vector.scalar_tensor_tensor(
            t2[:], ss[:], 1.0, t1[:],
            mybir.AluOpType.add, mybir.AluOpType.mult,
        )   # (1+ss)*sqrt(ss+eps)
        nc.vector.reciprocal(out=t2[:], in_=t2[:])
        nc.vector.scalar_tensor_tensor(
            t1[:], ss[:], inv, t2[:],
            mybir.AluOpType.mult, mybir.AluOpType.mult,
        )   # t1 = (ss*inv) / ((1+ss)*sqrt(ss+eps)) -- squash scale (*inv)
        if r == R - 1:
            # write the (j, d)-ordered output directly via a transposed AP
            out_t = small.tile([B, N_CAPS, OUT_DIM], f32)
            nc.vector.tensor_mul(
                out_t.transpose([0, 2, 1]), s_sb[:],
                t1.unsqueeze(1).broadcast_to([B, OUT_DIM, N_CAPS]),
            )
            nc.sync.dma_start(out[:], out_t[:])
        else:
            out_s = small.tile([B, OUT_DIM, N_CAPS], f32)
            nc.vector.tensor_mul(
                out_s[:], s_sb[:],
                t1.unsqueeze(1).broadcast_to([B, OUT_DIM, N_CAPS]),
            )
            nc.vector.tensor_copy(out=out2[0:B], in_=out_s[:])
            nc.sync.dma_start(out2[B:2 * B], out2[0:B])
            # bu-mul: broadcast outputs over middle i1 keeps bf16 2x mode
            nc.vector.tensor_mul(
                tmp[:], u_hat[:],
                out2.unsqueeze(2).broadcast_to([128, OUT_DIM, I1, N_CAPS]),
            )
            w = OUT_DIM
            while w > 2:
                nc.vector.tensor_add(
                    tmp[:, :w // 2], tmp[:, :w // 2], tmp[:, w // 2:w]
                )
                w //= 2
            bred = small.tile([128, I1, N_CAPS], f32)
            nc.vector.tensor_add(bred[:], tmp[:, 0], tmp[:, 1])
            nc.vector.tensor_add(b_log[:], b_log[:], bred[:])

    ctx.close()


def _build(num_routing):
    import concourse.bacc as bacc
    import concourse.tile as tile
    from concourse import mybir

    nc = bacc.Bacc(
        "TRN2", target_bir_lowering=False, debug=False, num_devices=N_CORES,
        dynamic_dma_scratch_size=512,
    )
    f32 = mybir.dt.float32
    bf16 = mybir.dt.bfloat16
    xT = nc.dram_tensor("xT", [IN_DIM, I_LOC, B], bf16, kind="ExternalInput")
    wT = nc.dram_tensor(
        "wT", [NGRP, IN_DIM, GRP, OUT_DIM, N_CAPS], bf16, kind="ExternalInput"
    )
    out = nc.dram_tensor(
        "out", [B, N_CAPS, OUT_DIM], f32, kind="ExternalOutput"
    )
    with tile.TileContext(nc) as tc:
        _emit(tc, xT, wT, out, num_routing)
    nc.compile()
    return nc


def kernel(inputs, W, num_routing):
    import ml_dtypes

    from concourse.bass_utils import run_bass_kernel_spmd

    R = int(num_routing)
    assert R >= 1
    if R not in _cache:
        _cache[R] = _build(R)
    nc = _cache[R]

    bf = ml_dtypes.bfloat16
    inputs = np.ascontiguousarray(np.asarray(inputs, dtype=np.float32))
    W = np.asarray(W, dtype=np.float32)

    in_maps = []
    for c in range(N_CORES):
        lo, hi = c * I_LOC, (c + 1) * I_LOC
        xT_c = np.ascontiguousarray(
            inputs[:, lo:hi, :].transpose(2, 1, 0).astype(bf)
        )
        # [i,j,k,d] -> group-blocked [g, k, t, d, j] so each group DMA is one
        # contiguous block and PSUM columns come out in (d, j) order
        wT_c = np.ascontiguousarray(
            W[lo:hi]
            .reshape(NGRP, GRP, N_CAPS, IN_DIM, OUT_DIM)
            .transpose(0, 3, 1, 4, 2)
            .astype(bf)
        )
        in_maps.append({"xT": xT_c, "wT": wT_c})

    kwargs = {}
    if TRACE:
        kwargs["trace"] = True
        if TRACE_DIR:
            kwargs["tmpdir"] = TRACE_DIR
    res = None
    for attempt in range(3):
        try:
            res = run_bass_kernel_spmd(
                nc, in_maps, core_ids=list(range(N_CORES)), **kwargs
            )
            break
        except Exception:
            if attempt == 2:
                raise
            import time
            time.sleep(5)
    if TRACE:
        kernel.last_exec_time_ns = res.exec_time_ns
        kernel.last_results = res
    return np.asarray(res.results[0]["out"], dtype=np.float32)


# revision 36
# speedup vs baseline: 1.1620x; 1.0242x over previous
"""CapsuleLayer (dynamic routing) Trainium2 kernel, SPMD over 8 NeuronCores.

Sharding: input-capsule axis (IN_CAPS=512 -> 64 per core). W and u_hat are
i-sharded; the bij,bijd->bjd contraction is completed with an AllReduce of
s-partials (2x64x16x32, 262 KB) once per routing iteration.

Per-core layout (i_local = i2*32 + i1, i2 in {0,1}):
  u_hat SBUF [p=(i2*64+b), (d, i1, j)] bf16 -- 128 partitions x 16384
  b/c logits [p, (i1, j)], s/outputs [b, (d, j)].

The (d, i1, j) free order keeps every big DVE pass in the bf16 2x perf mode:
both broadcast multiplies broadcast over a non-innermost dim (innermost stays
step-1), and both reductions are in-place contiguous tree-adds.

Phase 1 (per i): u_hat_i[b, dj] = xT_i.T @ W_i on the PE (K=128, M=64,
N=512), all in bf16 (x and W are cast host-side; u_hat is stored bf16
regardless, so the extra input rounding barely moves the final error, and the
shorter W stream lets the collective stack warm up sooner).

s partials are AllReduced per partition-half (engines cannot shift partitions;
the halves are summed after the AR instead of before).
"""

import numpy as np

N_CORES = 8
B = 64
IN_CAPS = 512
IN_DIM = 128
N_CAPS = 16
OUT_DIM = 32
I_LOC = IN_CAPS // N_CORES          # 64 input capsules per core
I1 = 32                             # i_local = i2*32 + i1
JD = N_CAPS * OUT_DIM               # 512
EPS = 1e-7
GRP = 4                             # i's per W-DMA/PSUM group
NGRP = I_LOC // GRP                 # 16

# Toggled by test.py for profiling runs.
TRACE = False
TRACE_DIR = None

_cache = {}


def _emit(tc, xT, wT, out, num_routing):
    from contextlib import ExitStack

    from concourse import mybir

    nc = tc.nc
    f32 = mybir.dt.float32
    bf16 = mybir.dt.bfloat16
    ctx = ExitStack()
    singles = ctx.enter_context(tc.tile_pool(name="singles", bufs=1))
    wpool = ctx.enter_context(tc.tile_pool(name="wpool", bufs=4))
    pspool = ctx.enter_context(tc.tile_pool(name="pspool", bufs=2, space="PSUM"))
    small = ctx.enter_context(tc.tile_pool(name="small", bufs=2))
    dram = ctx.enter_context(tc.tile_pool(name="dram", bufs=2, space="DRAM"))

    # One tiny warmup collective: the collective stack finishes its background
    # init ~60us into the kernel and charges a first-collective premium
    # (~12us); a 128-byte AllReduce absorbs both off the critical path.
    warm_in = dram.tile([1, 32], f32)
    warm_out = dram.tile([1, 32], f32)
    nc.gpsimd.collective_compute(
        "AllReduce",
        mybir.AluOpType.add,
        replica_groups=[list(range(N_CORES))],
        ins=[warm_in.opt()],
        outs=[warm_out.opt()],
    )

    # ---- phase 1: u_hat = einsum over k, per local capsule i ----
    xsb = singles.tile([IN_DIM, I_LOC, B], bf16)         # [k, i, b]
    u_hat = singles.tile([128, OUT_DIM, I1, N_CAPS], bf16)  # [(i2,b), d, i1, j]

    XCH = I_LOC // 4
    for g in range(NGRP):
        i2 = (g * GRP) // I1
        i1g = (g * GRP) % I1
        # interleave the x chunks with the first W groups so the first
        # matmul's operands land as early as possible
        if g < 4:
            q = g
            nc.sync.dma_start(
                xsb[:, q * XCH:(q + 1) * XCH, :],
                xT[:, q * XCH:(q + 1) * XCH, :],
            )
        wtile = wpool.tile([IN_DIM, GRP, OUT_DIM, N_CAPS], bf16)
        nc.sync.dma_start(wtile[:], wT[g])
        ps = pspool.tile([128, GRP, OUT_DIM, N_CAPS], f32)
        for t in range(GRP):
            i = g * GRP + t
            nc.tensor.matmul(
                ps[i2 * B:(i2 + 1) * B, t], xsb[:, i, :], wtile[:, t],
                start=True, stop=True,
            )
        # copy+cast PSUM f32 -> SBUF bf16; dst viewed (i1, d, j) to match src
        dst = u_hat[i2 * B:(i2 + 1) * B, :, i1g:i1g + GRP, :].transpose(
            [0, 2, 1, 3]
        )
        src = ps[i2 * B:(i2 + 1) * B]
        if g % 2 == 0:
            nc.vector.tensor_copy(out=dst, in_=src)
        else:
            nc.scalar.copy(out=dst, in_=src)

    # ---- phase 2: routing ----
    tmp = singles.tile([128, OUT_DIM, I1, N_CAPS], bf16)
    b_log = singles.tile([128, I1, N_CAPS], f32)
    out2 = singles.tile([128, OUT_DIM, N_CAPS], bf16)
    eps_t = singles.tile([B, 1], f32)
    nc.vector.memset(b_log[:], 0.0)
    nc.vector.memset(eps_t[:], EPS)

    R = num_routing
    for r in range(R):
        if r == 0:
            # b == 0 -> c uniform: s = (1/16) * sum_i u_hat (scale after AR)
            nc.vector.tensor_add(
                tmp[:, :, :I1 // 2], u_hat[:, :, :I1 // 2],
                u_hat[:, :, I1 // 2:],
            )
        else:
            # |b| stays < ~20 for this distribution: exp is fp32-safe
            # without the max-subtraction
            cexp = small.tile([128, I1, N_CAPS], f32)
            nc.scalar.activation(
                out=cexp[:], in_=b_log[:],
                func=mybir.ActivationFunctionType.Exp,
            )
            csum = small.tile([128, I1], f32)
            nc.vector.reduce_sum(
                out=csum[:], in_=cexp[:], axis=mybir.AxisListType.X
            )
            nc.vector.reciprocal(out=csum[:], in_=csum[:])
            c_t = small.tile([128, I1, N_CAPS], bf16)
            nc.vector.tensor_mul(
                c_t[:], cexp[:],
                csum.unsqueeze(2).broadcast_to([128, I1, N_CAPS]),
            )
            # s-mul: broadcast c over outermost d keeps bf16 2x mode
            nc.vector.tensor_mul(
                tmp[:], u_hat[:],
                c_t.unsqueeze(1).broadcast_to([128, OUT_DIM, I1, N_CAPS]),
            )
            nc.vector.tensor_add(
                tmp[:, :, :I1 // 2], tmp[:, :, :I1 // 2], tmp[:, :, I1 // 2:]
            )
        # contiguous in-place tree over i1 (middle dim); final level -> f32
        w = I1 // 2
        while w > 2:
            nc.vector.tensor_add(
                tmp[:, :, :w // 2], tmp[:, :, :w // 2], tmp[:, :, w // 2:w]
            )
            w //= 2
        s_half = small.tile([128, OUT_DIM, N_CAPS], bf16)
        nc.vector.tensor_add(s_half[:], tmp[:, :, 0, :], tmp[:, :, 1, :])

        # AllReduce both partition halves; sum them after (engines cannot
        # shift partitions, so pre-folding would need an extra DMA round)
        cc_in = dram.tile([2, B, OUT_DIM, N_CAPS], bf16)
        cc_out = dram.tile([2, B, OUT_DIM, N_CAPS], bf16)
        nc.sync.dma_start(cc_in[0], s_half[0:B])
        nc.sync.dma_start(cc_in[1], s_half[B:2 * B])
        nc.gpsimd.collective_compute(
            "AllReduce",
            mybir.AluOpType.add,
            replica_groups=[list(range(N_CORES))],
            ins=[cc_in.opt()],
            outs=[cc_out.opt()],
        )
        s_lo = small.tile([B, OUT_DIM, N_CAPS], bf16)
        s_hi = small.tile([B, OUT_DIM, N_CAPS], bf16)
        nc.gpsimd.dma_start(s_lo[:], cc_out[0])
        nc.gpsimd.dma_start(s_hi[:], cc_out[1])
        s_sb = small.tile([B, OUT_DIM, N_CAPS], f32)
        nc.vector.tensor_add(s_sb[:], s_lo[:], s_hi[:])
        # iteration 0's uniform c = 1/16 is folded into the squash math:
        # with s' = 16*s, ss = sum_d (s'/16)^2 and out = f(ss) * (s'/16)
        inv = 1.0 / N_CAPS if r == 0 else 1.0

        # squash: scale = ss/(1+ss)/sqrt(ss+eps), ss = sum_d s^2
        sq = small.tile([B, OUT_DIM, N_CAPS], f32)
        nc.vector.scalar_tensor_tensor(
            sq[:], s_sb[:], inv * inv, s_sb[:],
            mybir.AluOpType.mult, mybir.AluOpType.mult,
        )
        ss = small.tile([B, N_CAPS], f32)
        nc.vector.reduce_sum(
            out=ss[:], in_=sq.transpose([0, 2, 1]), axis=mybir.AxisListType.X
        )
        t1 = small.tile([B, N_CAPS], f32)
        nc.scalar.activation(
            out=t1[:], in_=ss[:], func=mybir.ActivationFunctionType.Sqrt,
            bias=eps_t[:], scale=1.0,
        )
        t2 = small.tile([B, N_CAPS], f32)
        nc.vector.scalar_tensor_tensor(
            t2[:], ss[:], 1.0, t1[:],
            mybir.AluOpType.add, mybir.AluOpType.mult,
        )   # (1+ss)*sqrt(ss+eps)
        nc.vector.reciprocal(out=t2[:], in_=t2[:])
        nc.vector.scalar_tensor_tensor(
            t1[:], ss[:], inv, t2[:],
            mybir.AluOpType.mult, mybir.AluOpType.mult,
        )   # t1 = (ss*inv) / ((1+ss)*sqrt(ss+eps)) -- squash scale (*inv)
        if r == R - 1:
            # write the (j, d)-ordered output directly via a transposed AP
            out_t = small.tile([B, N_CAPS, OUT_DIM], f32)
            nc.vector.tensor_mul(
                out_t.transpose([0, 2, 1]), s_sb[:],
                t1.unsqueeze(1).broadcast_to([B, OUT_DIM, N_CAPS]),
            )
            nc.sync.dma_start(out[:], out_t[:])
        else:
            nc.vector.tensor_mul(
                out2[0:B], s_sb[:],
                t1.unsqueeze(1).broadcast_to([B, OUT_DIM, N_CAPS]),
            )
            nc.sync.dma_start(out2[B:2 * B], out2[0:B])
            # bu-mul: broadcast outputs over middle i1 keeps bf16 2x mode
            nc.vector.tensor_mul(
                tmp[:], u_hat[:],
                out2.unsqueeze(2).broadcast_to([128, OUT_DIM, I1, N_CAPS]),
            )
            w = OUT_DIM
            while w > 2:
                nc.vector.tensor_add(
                    tmp[:, :w // 2], tmp[:, :w // 2], tmp[:, w // 2:w]
                )
                w //= 2
            bred = small.tile([128, I1, N_CAPS], f32)
            nc.vector.tensor_add(bred[:], tmp[:, 0], tmp[:, 1])
            nc.vector.tensor_add(b_log[:], b_log[:], bred[:])

    ctx.close()


def _build(num_routing):
    import concourse.bacc as bacc
    import concourse.tile as tile
    from concourse import mybir

    nc = bacc.Bacc(
        "TRN2", target_bir_lowering=False, debug=False, num_devices=N_CORES,
        dynamic_dma_scratch_size=512,
    )
    f32 = mybir.dt.float32
    bf16 = mybir.dt.bfloat16
    xT = nc.dram_tensor("xT", [IN_DIM, I_LOC, B], bf16, kind="ExternalInput")
    wT = nc.dram_tensor(
        "wT", [NGRP, IN_DIM, GRP, OUT_DIM, N_CAPS], bf16, kind="ExternalInput"
    )
    out = nc.dram_tensor(
        "out", [B, N_CAPS, OUT_DIM], f32, kind="ExternalOutput"
    )
    with tile.TileContext(nc) as tc:
        _emit(tc, xT, wT, out, num_routing)
    nc.compile()
    return nc


def kernel(inputs, W, num_routing):
    import ml_dtypes

    from concourse.bass_utils import run_bass_kernel_spmd

    R = int(num_routing)
    assert R >= 1
    if R not in _cache:
        _cache[R] = _build(R)
    nc = _cache[R]

    bf = ml_dtypes.bfloat16
    inputs = np.ascontiguousarray(np.asarray(inputs, dtype=np.float32))
    W = np.asarray(W, dtype=np.float32)

    in_maps = []
    for c in range(N_CORES):
        lo, hi = c * I_LOC, (c + 1) * I_LOC
        xT_c = np.ascontiguousarray(
            inputs[:, lo:hi, :].transpose(2, 1, 0).astype(bf)
        )
        # [i,j,k,d] -> group-blocked [g, k, t, d, j] so each group DMA is one
        # contiguous block and PSUM columns come out in (d, j) order
        wT_c = np.ascontiguousarray(
            W[lo:hi]
            .reshape(NGRP, GRP, N_CAPS, IN_DIM, OUT_DIM)
            .transpose(0, 3, 1, 4, 2)
            .astype(bf)
        )
        in_maps.append({"xT": xT_c, "wT": wT_c})

    kwargs = {}
    if TRACE:
        kwargs["trace"] = True
        if TRACE_DIR:
            kwargs["tmpdir"] = TRACE_DIR
    res = None
    for attempt in range(3):
        try:
            res = run_bass_kernel_spmd(
                nc, in_maps, core_ids=list(range(N_CORES)), **kwargs
            )
            break
        except Exception:
            if attempt == 2:
                raise
            import time
            time.sleep(5)
    if TRACE:
        kernel.last_exec_time_ns = res.exec_time_ns
        kernel.last_results = res
    return np.asarray(res.results[0]["out"], dtype=np.float32)
